# revision 38
# baseline (speedup 1.0000x reference)
"""Self-contained Trainium2 Bass kernel for 4-layer GraphSAGE (nn_LASAGE).

Strategy (v4 — fp8 tables, host one-hots, 3-chunk pipelined AllGathers):
  - Nodes dst-sharded across 8 cores (6250/core, padded to 6272 = 49 blocks of 128).
  - Aggregation is done POST-matmul: agg(x)@Wl == agg(x@Wl), so per layer each
    core computes y = h @ Wl for its own shard; the full Y table [50176, d] is
    replicated via THREE chunked AllGathers (blocks [0:17) [17:33) [33:49)),
    kicked as soon as each chunk's rows are written. Chunk tables stay under
    the int16 idx limit (17408 rows). Edges gather y[src] rows with dma_gather
    (fp8e4, 256B rows) on 4 SWDGE queues.
  - Chunk-0/1 gather calls of the first W blocks are PREFLIGHTED at layer
    start: their AGs completed mid-previous-layer, so they fill the DMA
    engines while the previous layer's tail chunk-2 AllGather (only 10/49 of
    a table) is still landing. Gather calls are capped at 768 idxs: >768
    crashes or falls off a ucode performance cliff (1024 runs 50x slower).
  - Scatter-add into dst blocks via one-hot matmuls on the PE. The one-hot is
    UNSCALED {0,1} fp8, built per block with a single DVE IS_EQ against a
    host-provided bf16 dst-column map. Mean-normalization moves to the edges:
      out = invd[dst] * (gather_sum + degc[dst]*(x@Wr) + degc[dst]*b)
    with degc = max(deg,1) pre-scaled Wr inputs and an invd epilogue
    (ACT relu + DVE column-scale for L1/L2's transposed psum; a single ACT
    Copy with per-partition scale for L3's [dst, feat] psum).
  - All dense operands (x, weights, h storage) are bf16; psum stays fp32.
  - Layer1 fuses conv0+conv1 (concat -> 256 feat). Layer3 (output, d=64) uses
    non-transposed psum (lhsT=onehot) so rows DMA straight to the output;
    its fp8 table rows are 256-wide with only cols 0:64 valid.
"""
import sys, os, types

sys.path.insert(0, "/opt/trn_rl_repo")
import numpy as np

N = 50000
E = 800000
NCORES = 8
S = N // NCORES            # 6250 real nodes per core
SP = 6272                  # padded (49 blocks of 128)
NBLK = SP // 128
D1 = 256                   # concat(h0, h1)
DM = 256
DO = 64
MAXI = 768                 # max idxs per dma_gather call
TPC = MAXI // 128          # tiles per full call (6)
CPC = MAXI // 16           # idx-image cols per call (48)
MT = 9                     # max tiles per (chunk, dst-block) segment
NCH = 3
CBLK = [0, 22, 39, 49]     # chunk boundaries in blocks (small tail AG)
CST = [b * 128 for b in CBLK[:-1]]              # chunk start rows (per core)
CSZ = [(CBLK[i + 1] - CBLK[i]) * 128 for i in range(NCH)]   # [2176, 2048, 2048]
TBL = [NCORES * s for s in CSZ]                 # AG table rows (int16-safe)


def _install_hooks():
    """antenv.axon_hooks shim so trace=True works in this image (optional)."""
    try:
        import antenv
        if "antenv.axon_hooks" not in sys.modules:
            mod = types.ModuleType("antenv.axon_hooks")
            mod._hook = None
            mod.set_axon_ntff_profile_hook = lambda h: setattr(mod, "_hook", h)
            mod.get_axon_ntff_profile_hook = lambda: mod._hook
            sys.modules["antenv.axon_hooks"] = mod
            antenv.axon_hooks = mod
        from antenv.axon_hooks import get_axon_ntff_profile_hook, set_axon_ntff_profile_hook
        if get_axon_ntff_profile_hook() is None:
            from trn_agent_boot.trn_boot import _ntff_profile_via_ctypes
            set_axon_ntff_profile_hook(_ntff_profile_via_ctypes("/opt/axon/libaxon_pjrt.so"))
        import concourse.bass_utils as bu
        bu.upload_artifacts = lambda tmpdir: f"file://{tmpdir}"
    except Exception:
        pass


def _preprocess(edge_index):
    """Edge lists per core, grouped by (dst block, src chunk), padded per-tile."""
    src = np.asarray(edge_index[0], np.int64)
    dst = np.asarray(edge_index[1], np.int64)
    core = dst // S
    dl = (dst % S).astype(np.int64)
    blk = dl // 128
    col = dl % 128
    sloc = src % S
    chunk = np.digitize(sloc, [CST[1], CST[2]])
    cst = np.asarray(CST)[chunk]
    csz = np.asarray(CSZ)[chunk]
    grow = (src // S) * csz + (sloc - cst)   # row within its chunk-table

    deg = np.bincount(core * S + dl, minlength=N).reshape(NCORES, S)

    order = np.lexsort((grow, blk, chunk, core))
    core_s, ch_s, blk_s, col_s, row_s = (core[order], chunk[order], blk[order],
                                         col[order], grow[order])

    key = (core_s * NCH + ch_s) * NBLK + blk_s
    counts = np.bincount(key, minlength=NCORES * NCH * NBLK).reshape(NCORES, NCH, NBLK)
    tiles_hb = np.ceil(counts.max(axis=0) / 128).astype(np.int64)   # [NCH, NBLK]
    tiles_hb = np.maximum(tiles_hb, 1)

    pad_hb = tiles_hb * 128
    tot_h = pad_hb.sum(axis=1)
    seg_off = np.zeros((NCH, NBLK), np.int64)
    seg_off[:, 1:] = np.cumsum(pad_hb, axis=1)[:, :-1]

    srcpad = np.zeros((NCORES, NCH), dtype=object)
    colpad = np.zeros((NCORES, NCH), dtype=object)
    for c in range(NCORES):
        for h in range(NCH):
            srcpad[c, h] = np.zeros(int(tot_h[h]), np.int64)
            colpad[c, h] = np.full(int(tot_h[h]), -1, np.int64)
    grp = key
    first = np.r_[True, grp[1:] != grp[:-1]]
    gidx = np.arange(len(grp)) - np.maximum.accumulate(np.where(first, np.arange(len(grp)), 0))
    pos = seg_off[ch_s, blk_s] + gidx
    for c in range(NCORES):
        m = core_s == c
        for h in range(NCH):
            mh = m & (ch_s == h)
            p = pos[mh]
            srcpad[c, h][p] = row_s[mh]
            colpad[c, h][p] = col_s[mh]

    return {
        "tiles_hb": tiles_hb, "seg_off": seg_off,
        "srcpad": srcpad, "colpad": colpad, "deg": deg,
    }


def _build_callplan(tiles_hb):
    """Gather call plan (compile-time, same for every core)."""
    calls = []
    block_calls = {b: {h: [] for h in range(NCH)} for b in range(NBLK)}
    block_tiles = {b: [] for b in range(NBLK)}
    tile_ctr = [0] * NCH
    ht_base = np.concatenate([[0], np.cumsum(tiles_hb.sum(axis=1))])[:NCH]
    for b in range(NBLK):
        for h in range(NCH):
            nt = int(tiles_hb[h, b])
            done = 0
            while done < nt:
                k = min(TPC, nt - done)
                ci = len(calls)
                calls.append(dict(h=h, k=k, tile_base=tile_ctr[h], blk=b, q=0))
                block_calls[b][h].append(ci)
                for j in range(k):
                    dcol = int(ht_base[h]) + tile_ctr[h] + j
                    block_tiles[b].append((ci, j, dcol))
                tile_ctr[h] += k
                done += k
    qload = [0, 0, 0, 0]
    for cl in calls:
        q = qload.index(min(qload))
        cl["q"] = q
        qload[q] += cl["k"]
    return calls, block_calls, block_tiles


def _blk_oh_ranges(tiles_hb):
    """Per block, per chunk: (first dcol, ntiles) of its one-hot image range."""
    ht_base = np.concatenate([[0], np.cumsum(tiles_hb.sum(axis=1))])[:NCH]
    out = []
    for b in range(NBLK):
        r = []
        for h in range(NCH):
            start = int(ht_base[h]) + int(tiles_hb[h, :b].sum())
            r.append((start, int(tiles_hb[h, b])))
        out.append(r)
    return out


def _idx_arrays(pre, calls, core):
    """int16 idx image [128, ncalls*CPC] and one-hot image [128, tiles*128] f8."""
    import ml_dtypes as _ml
    ncalls = len(calls)
    idx_img = np.zeros((16, ncalls * CPC), np.int16)
    tiles_total = int(pre["tiles_hb"].sum())
    dstloc = np.full((128, tiles_total), -1, np.int64)
    ht_base = np.concatenate([[0], np.cumsum(pre["tiles_hb"].sum(axis=1))])[:NCH]
    for ci, cl in enumerate(calls):
        h, k, tb, b = cl["h"], cl["k"], cl["tile_base"], cl["blk"]
        e0 = int(pre["seg_off"][h, b]) + (tb - int(pre["tiles_hb"][h, :b].sum())) * 128
        nidx = k * 128
        seg_src = pre["srcpad"][core, h][e0:e0 + nidx]
        seg_col = pre["colpad"][core, h][e0:e0 + nidx]
        idx_img[:, ci * CPC: ci * CPC + (nidx // 16)] = seg_src.reshape(-1, 16).T.astype(np.int16)
        for t in range(k):
            dcol = int(ht_base[h]) + tb + t
            dstloc[:, dcol] = seg_col[t * 128:(t + 1) * 128]
    return np.tile(idx_img, (8, 1)), dstloc.astype(np.float32).astype(_ml.bfloat16)


def _build_bass(pre, calls, block_calls, block_tiles, ncalls_cols, tiles_total):
    import concourse.bass as bass
    import concourse.bacc as bacc
    import concourse.mybir as mybir
    import concourse.tile as tile

    FP32 = mybir.dt.float32
    BF16 = mybir.dt.bfloat16
    F8 = mybir.dt.float8e4
    I16 = mybir.dt.int16
    AL = mybir.AluOpType
    AF = mybir.ActivationFunctionType

    blk_oh = _blk_oh_ranges(pre["tiles_hb"])

    nc = bacc.Bacc("TRN2", target_bir_lowering=False, debug=False,
                   enable_asserts=False, num_devices=NCORES, num_swdge_queues=4)

    x0T = nc.dram_tensor("x0T", [128, SP], BF16, kind="ExternalInput")
    x1T = nc.dram_tensor("x1T", [128, SP], BF16, kind="ExternalInput")
    x0dT = nc.dram_tensor("x0dT", [128, SP], BF16, kind="ExternalInput")
    x1dT = nc.dram_tensor("x1dT", [128, SP], BF16, kind="ExternalInput")
    wl0 = nc.dram_tensor("wl0", [128, 128], BF16, kind="ExternalInput")
    wr0 = nc.dram_tensor("wr0", [128, 128], BF16, kind="ExternalInput")
    wl1 = nc.dram_tensor("wl1", [128, 128], BF16, kind="ExternalInput")
    wr1 = nc.dram_tensor("wr1", [128, 128], BF16, kind="ExternalInput")
    wlm = nc.dram_tensor("wlm", [256, 256], BF16, kind="ExternalInput")
    wrm = nc.dram_tensor("wrm", [256, 256], BF16, kind="ExternalInput")
    wlo = nc.dram_tensor("wlo", [256, 64], BF16, kind="ExternalInput")
    wro = nc.dram_tensor("wro", [256, 64], BF16, kind="ExternalInput")
    b01d = nc.dram_tensor("b01", [1, 256], BF16, kind="ExternalInput")
    bmd = nc.dram_tensor("bm", [1, 256], BF16, kind="ExternalInput")
    bod = nc.dram_tensor("bo", [1, 64], BF16, kind="ExternalInput")
    idxd = nc.dram_tensor("idx", [128, ncalls_cols], I16, kind="ExternalInput")
    dstld = nc.dram_tensor("dstl", [128, tiles_total], BF16, kind="ExternalInput")
    invrd = nc.dram_tensor("invr", [128, SP], BF16, kind="ExternalInput")
    degrd = nc.dram_tensor("degr", [128, SP], BF16, kind="ExternalInput")
    invcd = nc.dram_tensor("invc", [128, NBLK], FP32, kind="ExternalInput")
    outd = nc.dram_tensor("out", [S, DO], FP32, kind="ExternalOutput")

    with tile.TileContext(nc) as tc:
        with (
            tc.tile_pool(name="const", bufs=1) as cp,
            tc.tile_pool(name="acts", bufs=1) as hp,
            tc.tile_pool(name="g", bufs=28) as gp,
            tc.tile_pool(name="oh", bufs=6) as ohp,
            tc.tile_pool(name="xs", bufs=6) as xsp,
            # PSUM budget (8 banks): ps0/ps1 (3 bufs each = 6 banks) + py (2)
            tc.tile_pool(name="ps", bufs=3, space="PSUM") as psp,
            tc.tile_pool(name="psy", bufs=2, space="PSUM") as psyp,
            tc.tile_pool(name="ev", bufs=4) as evp,
            tc.tile_pool(name="dram", bufs=1, space="DRAM") as dp,
        ):
            def load(name, dt_, shape, src):
                t = cp.tile(shape, dt_, name=name)
                nc.sync.dma_start(out=t[:], in_=src)
                return t

            wl0t = load("wl0t", BF16, [128, 128], wl0[:])
            wr0t = load("wr0t", BF16, [128, 128], wr0[:])
            wl1t = load("wl1t", BF16, [128, 128], wl1[:])
            wr1t = load("wr1t", BF16, [128, 128], wr1[:])
            wlmt = [load(f"wlmt{i}", BF16, [128, 256], wlm[i * 128:(i + 1) * 128, :]) for i in range(2)]
            wrmt = [load(f"wrmt{i}", BF16, [128, 256], wrm[i * 128:(i + 1) * 128, :]) for i in range(2)]
            wlot = [load(f"wlot{i}", BF16, [128, 64], wlo[i * 128:(i + 1) * 128, :]) for i in range(2)]
            wrot = [load(f"wrot{i}", BF16, [128, 64], wro[i * 128:(i + 1) * 128, :]) for i in range(2)]
            b01t = load("b01t", BF16, [1, 256], b01d[:])
            bmt = load("bmt", BF16, [1, 256], bmd[:])
            bot = load("bot", BF16, [1, 64], bod[:])
            idxt = load("idxt", I16, [128, ncalls_cols], idxd[:])
            invr = load("invrt", BF16, [128, SP], invrd[:])
            degr = load("degrt", BF16, [128, SP], degrd[:])
            invc = load("invct", FP32, [128, NBLK], invcd[:])
            dstl = load("dstlt", BF16, [128, tiles_total], dstld[:])

            iota_i = cp.tile([128, MT, 128], mybir.dt.int32, name="iota_i")
            nc.gpsimd.iota(iota_i[:], pattern=[[0, MT], [1, 128]], base=0,
                           channel_multiplier=0)
            iota_bf = cp.tile([128, MT, 128], BF16, name="iota_bf")
            nc.vector.tensor_copy(out=iota_bf[:], in_=iota_i[:])

            # memset gather pool once: padded idx slots gather row 0 (finite),
            # keeping every slot's stale data finite for zero one-hot columns.
            for i in range(28):
                gz = gp.tile([128, TPC, D1], F8, name="gz", tag="g")
                nc.vector.memset(gz[:], 0.0)

            warm_own = dp.tile([8, 256], F8, name="warm_own")
            warm_tab = dp.tile([64, 256], F8, name="warm_tab",
                               addr_space="Shared" if NCORES > 4 else "Local")
            wz = evp.tile([8, 256], F8, name="wz", tag="wz")
            nc.vector.memset(wz[:], 0.0)
            nc.sync.dma_start(out=warm_own[:], in_=wz[:])
            nc.gpsimd.collective_compute(
                "AllGather", AL.bypass, replica_groups=[list(range(NCORES))],
                ins=[warm_own[:]], outs=[warm_tab[:]])

            hT = [hp.tile([128, SP], BF16, name=f"hT{i}") for i in range(2)]
            h2T = [hp.tile([128, SP], BF16, name=f"h2T{i}") for i in range(2)]

            shared = "Shared" if NCORES > 4 else "Local"

            def mk_tables(name, width):
                own = [dp.tile([CSZ[h], width], F8, name=f"{name}_own{h}")
                       for h in range(NCH)]
                tab = [dp.tile([TBL[h], width], F8, name=f"{name}{h}",
                               addr_space=shared) for h in range(NCH)]
                return own, tab

            y01_own, Y01 = mk_tables("y01", D1)
            ym_own, Ym = mk_tables("ym", DM)
            yo_own, Yo = mk_tables("yo", 256)

            def chunk_of_block(b):
                return 0 if b < CBLK[1] else (1 if b < CBLK[2] else 2)

            def write_y(dsts, b, src_tile, dcols):
                h = chunk_of_block(b)
                r0 = b * 128 - CST[h]
                nc.sync.dma_start(out=dsts[h][r0:r0 + 128, 0:dcols],
                                  in_=src_tile[:, 0:dcols])

            RG = [list(range(NCORES))]

            def blk_sl(b):
                return slice(b * 128, (b + 1) * 128)

            def make_ags(own, tab):
                def mk(h):
                    def f():
                        nc.gpsimd.collective_compute(
                            "AllGather", AL.bypass, replica_groups=RG,
                            ins=[own[h][:]], outs=[tab[h][:]])
                    return f
                return [mk(h) for h in range(NCH)]

            def load_oh(b):
                """Build this block's one-hot tiles with a single DVE IS_EQ."""
                tiles = {}
                for h in range(NCH):
                    start, nt = blk_oh[b][h]
                    t = ohp.tile([128, MT, 128], F8, name=f"ohb{h}", tag=f"oh{h}")
                    nc.vector.tensor_tensor(
                        out=t[:, 0:nt, :], in0=iota_bf[:, 0:nt, :],
                        in1=dstl[:, start:start + nt].to_broadcast([128, nt, 128]),
                        op=AL.is_equal)
                    tiles[h] = (t, start)
                return tiles

            # AG kick: chunk i kicks a few blocks after its rows are written
            # so the kick's input-wait is already satisfied; tail at loop end.
            ag_at = {CBLK[1] + 3: 0, CBLK[2] + 1: 1, CBLK[3] - 1: 2}

            # ================= L1 pre: y01_own = [x0@Wl0 | x1@Wl1] =========
            ags01 = make_ags(y01_own, Y01)
            ag_at_pre = {CBLK[1] - 1: 0, CBLK[2] - 1: 1, CBLK[3] - 1: 2}
            for b in range(NBLK):
                x0b = xsp.tile([128, 128], BF16, name="x0b", tag="x0b")
                nc.sync.dma_start(out=x0b[:], in_=x0T[:, blk_sl(b)])
                x1b = xsp.tile([128, 128], BF16, name="x1b", tag="x1b")
                nc.sync.dma_start(out=x1b[:], in_=x1T[:, blk_sl(b)])
                py0 = psp.tile([128, 128], FP32, name="py0", tag="ps0")
                py1 = psp.tile([128, 128], FP32, name="py1", tag="ps1")
                nc.tensor.matmul(py0[:], lhsT=x0b[:], rhs=wl0t[:], start=True, stop=True)
                nc.tensor.matmul(py1[:], lhsT=x1b[:], rhs=wl1t[:], start=True, stop=True)
                evy = evp.tile([128, 256], F8, name="evy", tag="evy", padded_shape=[128, 512])
                nc.vector.tensor_copy(out=evy[:, 0:128], in_=py0[:])
                nc.vector.tensor_copy(out=evy[:, 128:256], in_=py1[:])
                write_y(y01_own, b, evy, D1)
                if b in ag_at_pre:
                    ags01[ag_at_pre[b]]()

            # ================= aggregation layer (L1/L2) =====================
            def agg_layer(Ytab, wr_tiles, bias_t, h_src, h_dst, wl_next, y_next,
                          d_next, ags_next):
                gtiles = {}
                qn = [0]

                def emit_gathers(cis):
                    for ci in cis:
                        cl = calls[ci]
                        k = cl["k"]
                        g = gp.tile([128, TPC, D1], F8, name="g", tag="g")
                        nc.gpsimd.dma_gather(
                            out_ap=g[:, 0:k, :],
                            in_ap=Ytab[cl["h"]][:],
                            idxs_ap=idxt[:, ci * CPC: ci * CPC + (k * 128) // 16],
                            num_idxs=k * 128, num_idxs_reg=k * 128,
                            elem_size=D1, queue_num=cl["q"])
                        gtiles[ci] = g

                W = 7
                for b in range(W):
                    emit_gathers(block_calls[b][0])
                    emit_gathers(block_calls[b][1])
                for b in range(NBLK):
                    emit_gathers(block_calls[b][2])
                    if b + W < NBLK:
                        emit_gathers(block_calls[b + W][0])
                        emit_gathers(block_calls[b + W][1])
                    ohb = load_oh(b)
                    ps0 = psp.tile([128, 128], FP32, name="ps0", tag="ps0")
                    ps1 = psp.tile([128, 128], FP32, name="ps1", tag="ps1")
                    if h_src is None:
                        x0b = xsp.tile([128, 128], BF16, name="x0b2", tag="xd0")
                        nc.sync.dma_start(out=x0b[:], in_=x0dT[:, blk_sl(b)])
                        x1b = xsp.tile([128, 128], BF16, name="x1b2", tag="xd1")
                        nc.sync.dma_start(out=x1b[:], in_=x1dT[:, blk_sl(b)])
                        nc.tensor.matmul(ps0[:], lhsT=wr0t[:], rhs=x0b[:], start=True, stop=False)
                        nc.tensor.matmul(ps1[:], lhsT=wr1t[:], rhs=x1b[:], start=True, stop=False)
                    else:
                        hd0 = evp.tile([128, 128], BF16, name="hd0", tag="hd0")
                        nc.vector.tensor_tensor(out=hd0[:], in0=h_src[0][:, blk_sl(b)],
                                                in1=degr[:, blk_sl(b)], op=AL.mult)
                        hd1 = evp.tile([128, 128], BF16, name="hd1", tag="hd1")
                        nc.vector.tensor_tensor(out=hd1[:], in0=h_src[1][:, blk_sl(b)],
                                                in1=degr[:, blk_sl(b)], op=AL.mult)
                        nc.tensor.matmul(ps0[:], lhsT=wr_tiles[0][:, 0:128], rhs=hd0[:], start=True, stop=False)
                        nc.tensor.matmul(ps0[:], lhsT=wr_tiles[1][:, 0:128], rhs=hd1[:], start=False, stop=False)
                        nc.tensor.matmul(ps1[:], lhsT=wr_tiles[0][:, 128:256], rhs=hd0[:], start=True, stop=False)
                        nc.tensor.matmul(ps1[:], lhsT=wr_tiles[1][:, 128:256], rhs=hd1[:], start=False, stop=False)
                    nc.tensor.matmul(ps0[:], lhsT=bias_t[0:1, 0:128], rhs=degr[0:1, blk_sl(b)],
                                     start=False, stop=False)
                    nc.tensor.matmul(ps1[:], lhsT=bias_t[0:1, 128:256], rhs=degr[0:1, blk_sl(b)],
                                     start=False, stop=False)
                    tl = block_tiles[b]
                    for n, (ci, slot, dcol) in enumerate(tl):
                        g = gtiles[ci]
                        oht, start = ohb[calls[ci]["h"]]
                        j = dcol - start
                        last = (n == len(tl) - 1)
                        nc.tensor.matmul(ps0[:], lhsT=g[:, slot, 0:128], rhs=oht[:, j, :],
                                         start=False, stop=last)
                        nc.tensor.matmul(ps1[:], lhsT=g[:, slot, 128:256], rhs=oht[:, j, :],
                                         start=False, stop=last)
                    # epilogue: h = relu(ps) * invd  (relu commutes with the
                    # positive per-column scale)
                    rt0 = evp.tile([128, 128], BF16, name="rt0", tag="rt0")
                    nc.scalar.activation(rt0[:], ps0[:], AF.Relu)
                    nc.vector.tensor_tensor(out=h_dst[0][:, blk_sl(b)], in0=rt0[:],
                                            in1=invr[:, blk_sl(b)], op=AL.mult)
                    rt1 = evp.tile([128, 128], BF16, name="rt1", tag="rt1")
                    nc.scalar.activation(rt1[:], ps1[:], AF.Relu)
                    nc.vector.tensor_tensor(out=h_dst[1][:, blk_sl(b)], in0=rt1[:],
                                            in1=invr[:, blk_sl(b)], op=AL.mult)
                    pyn = psyp.tile([128, d_next], FP32, name="pyn", tag="py",
                                    padded_shape=[128, 256])
                    nc.tensor.matmul(pyn[:], lhsT=h_dst[0][:, blk_sl(b)], rhs=wl_next[0][:],
                                     start=True, stop=False)
                    nc.tensor.matmul(pyn[:], lhsT=h_dst[1][:, blk_sl(b)], rhs=wl_next[1][:],
                                     start=False, stop=True)
                    evn = evp.tile([128, d_next], F8, name="evn", tag="evy",
                                   padded_shape=[128, 512])
                    nc.vector.tensor_copy(out=evn[:], in_=pyn[:])
                    write_y(y_next, b, evn, d_next)
                    if b in ag_at:
                        ags_next[ag_at[b]]()

            agg_layer(Y01, None, b01t, None, hT, wlmt, ym_own, DM,
                      make_ags(ym_own, Ym))
            agg_layer(Ym, wrmt, bmt, hT, h2T, wlot, yo_own, DO,
                      make_ags(yo_own, Yo))

            # ================= L3: out[node, 64] ============================
            qn3 = [0]
            gtiles3 = {}

            def emit_gathers3(cis):
                for ci in cis:
                    cl = calls[ci]
                    k = cl["k"]
                    g3 = gp.tile([128, TPC, 256], F8, name="g3", tag="g")
                    nc.gpsimd.dma_gather(
                        out_ap=g3[:, 0:k, :], in_ap=Yo[cl["h"]][:],
                        idxs_ap=idxt[:, ci * CPC: ci * CPC + (k * 128) // 16],
                        num_idxs=k * 128, num_idxs_reg=k * 128,
                        elem_size=256, queue_num=cl["q"])
                    gtiles3[ci] = g3

            W3 = 7
            for b in range(W3):
                emit_gathers3(block_calls[b][0])
                emit_gathers3(block_calls[b][1])
            for b in range(NBLK):
                emit_gathers3(block_calls[b][2])
                if b + W3 < NBLK:
                    emit_gathers3(block_calls[b + W3][0])
                    emit_gathers3(block_calls[b + W3][1])
                ohb = load_oh(b)
                ps3 = psp.tile([128, DO], FP32, name="ps3", tag="ps0",
                               padded_shape=[128, 128])
                hd0 = evp.tile([128, 128], BF16, name="hd20", tag="hd0")
                nc.vector.tensor_tensor(out=hd0[:], in0=h2T[0][:, blk_sl(b)],
                                        in1=degr[:, blk_sl(b)], op=AL.mult)
                hd1 = evp.tile([128, 128], BF16, name="hd21", tag="hd1")
                nc.vector.tensor_tensor(out=hd1[:], in0=h2T[1][:, blk_sl(b)],
                                        in1=degr[:, blk_sl(b)], op=AL.mult)
                nc.tensor.matmul(ps3[:], lhsT=hd0[:], rhs=wrot[0][:],
                                 start=True, stop=False)
                nc.tensor.matmul(ps3[:], lhsT=hd1[:], rhs=wrot[1][:],
                                 start=False, stop=False)
                nc.tensor.matmul(ps3[:], lhsT=degr[0:1, blk_sl(b)], rhs=bot[0:1, :],
                                 start=False, stop=False)
                tl = block_tiles[b]
                for n, (ci, slot, dcol) in enumerate(tl):
                    g3 = gtiles3[ci]
                    oht, start = ohb[calls[ci]["h"]]
                    j = dcol - start
                    nc.tensor.matmul(ps3[:], lhsT=oht[:, j, :], rhs=g3[:, slot, 0:64],
                                     start=False, stop=(n == len(tl) - 1))
                osb = evp.tile([128, DO], FP32, name="osb", tag="osb")
                nc.scalar.activation(osb[:], ps3[:], AF.Copy,
                                     scale=invc[:, b:b + 1])
                rows = min(128, S - b * 128)
                nc.sync.dma_start(out=outd[b * 128: b * 128 + rows, :],
                                  in_=osb[0:rows, :])

    nc.finalize()
    return nc


_CACHE = {}


def _make_inmaps(inputs, pre, calls):
    import ml_dtypes as _ml
    BF = _ml.bfloat16
    x0 = np.asarray(inputs["x0"], np.float32)
    x1 = np.asarray(inputs["x1"], np.float32)
    deg = pre["deg"]
    bf16 = lambda a: np.ascontiguousarray(a).astype(BF)
    in_maps = []
    for c in range(NCORES):
        degc = np.maximum(deg[c], 1.0).astype(np.float32)
        invd = (1.0 / degc).astype(np.float32)
        degc_p = np.ones(SP, np.float32)
        degc_p[:S] = degc
        invd_p = np.ones(SP, np.float32)
        invd_p[:S] = invd
        idx_img, dstloc = _idx_arrays(pre, calls, c)
        x0c = np.zeros((128, SP), np.float32)
        x0c[:, :S] = x0[c * S:(c + 1) * S, :].T
        x1c = np.zeros((128, SP), np.float32)
        x1c[:, :S] = x1[c * S:(c + 1) * S, :].T
        x0dc = x0c * degc_p[None, :]
        x1dc = x1c * degc_p[None, :]
        in_maps.append({
            "x0T": bf16(x0c), "x1T": bf16(x1c),
            "x0dT": bf16(x0dc), "x1dT": bf16(x1dc),
            "wl0": bf16(inputs["Wl0"]), "wr0": bf16(inputs["Wr0"]),
            "wl1": bf16(inputs["Wl1"]), "wr1": bf16(inputs["Wr1"]),
            "wlm": bf16(inputs["Wlm"]), "wrm": bf16(inputs["Wrm"]),
            "wlo": bf16(inputs["Wlo"]), "wro": bf16(inputs["Wro"]),
            "b01": bf16(np.concatenate([np.asarray(inputs["b0"], np.float32),
                                        np.asarray(inputs["b1"], np.float32)])[None, :]),
            "bm": bf16(np.asarray(inputs["bm"], np.float32)[None, :]),
            "bo": bf16(np.asarray(inputs["bo"], np.float32)[None, :]),
            "idx": idx_img, "dstl": dstloc,
            "invr": bf16(np.broadcast_to(invd_p[None, :], (128, SP))),
            "degr": bf16(np.broadcast_to(degc_p[None, :], (128, SP))),
            "invc": np.ascontiguousarray(invd_p[:NBLK * 128].reshape(NBLK, 128).T,
                                         np.float32),
        })
    return in_maps


def _get_program(edge_index):
    if "prog" in _CACHE:
        return _CACHE["prog"]
    pre = _preprocess(edge_index)
    calls, block_calls, block_tiles = _build_callplan(pre["tiles_hb"])
    tiles_total = int(pre["tiles_hb"].sum())
    nc = _build_bass(pre, calls, block_calls, block_tiles, len(calls) * CPC, tiles_total)
    _CACHE["prog"] = (nc, pre, calls)
    return _CACHE["prog"]


LAST_EXEC_NS = None


def kernel(**inputs):
    global LAST_EXEC_NS
    _install_hooks()
    from concourse.bass_utils import run_bass_kernel_spmd

    nc, pre, calls = _get_program(inputs["edge_index"])
    in_maps = _make_inmaps(inputs, pre, calls)
    trace = os.environ.get("KERNEL_TRACE", "0") == "1"
    res = run_bass_kernel_spmd(nc, in_maps, list(range(NCORES)), trace=trace)
    LAST_EXEC_NS = res.exec_time_ns
    return np.concatenate([np.asarray(res.results[c]["out"]) for c in range(NCORES)], axis=0)


# revision 39
# speedup vs baseline: 1.0083x; 1.0083x over previous
"""Self-contained Trainium2 Bass kernel for 4-layer GraphSAGE (nn_LASAGE).

Strategy (v4 — fp8 tables, host one-hots, 3-chunk pipelined AllGathers):
  - Nodes dst-sharded across 8 cores (6250/core, padded to 6272 = 49 blocks of 128).
  - Aggregation is done POST-matmul: agg(x)@Wl == agg(x@Wl), so per layer each
    core computes y = h @ Wl for its own shard; the full Y table [50176, d] is
    replicated via THREE chunked AllGathers (blocks [0:17) [17:33) [33:49)),
    kicked as soon as each chunk's rows are written. Chunk tables stay under
    the int16 idx limit (17408 rows). Edges gather y[src] rows with dma_gather
    (fp8e4, 256B rows) on 4 SWDGE queues.
  - Chunk-0/1 gather calls of the first W blocks are PREFLIGHTED at layer
    start: their AGs completed mid-previous-layer, so they fill the DMA
    engines while the previous layer's tail chunk-2 AllGather (only 10/49 of
    a table) is still landing. Gather calls are capped at 768 idxs: >768
    crashes or falls off a ucode performance cliff (1024 runs 50x slower).
  - Scatter-add into dst blocks via one-hot matmuls on the PE. The one-hot is
    UNSCALED {0,1} fp8, built per block with a single DVE IS_EQ against a
    host-provided bf16 dst-column map. Mean-normalization moves to the edges:
      out = invd[dst] * (gather_sum + degc[dst]*(x@Wr) + degc[dst]*b)
    with degc = max(deg,1) pre-scaled Wr inputs and an invd epilogue
    (ACT relu + DVE column-scale for L1/L2's transposed psum; a single ACT
    Copy with per-partition scale for L3's [dst, feat] psum).
  - All dense operands (x, weights, h storage) are bf16; psum stays fp32.
  - Layer1 fuses conv0+conv1 (concat -> 256 feat). Layer3 (output, d=64) uses
    non-transposed psum (lhsT=onehot) so rows DMA straight to the output;
    its fp8 table rows are 256-wide with only cols 0:64 valid.
"""
import sys, os, types

sys.path.insert(0, "/opt/trn_rl_repo")
import numpy as np

N = 50000
E = 800000
NCORES = 8
S = N // NCORES            # 6250 real nodes per core
SP = 6272                  # padded (49 blocks of 128)
NBLK = SP // 128
D1 = 256                   # concat(h0, h1)
DM = 256
DO = 64
MAXI = 768                 # max idxs per dma_gather call
TPC = MAXI // 128          # tiles per full call (6)
CPC = MAXI // 16           # idx-image cols per call (48)
MT = 9                     # max tiles per (chunk, dst-block) segment
NCH = 3
CBLK = [0, 22, 39, 49]     # chunk boundaries in blocks (small tail AG)
CST = [b * 128 for b in CBLK[:-1]]              # chunk start rows (per core)
CSZ = [(CBLK[i + 1] - CBLK[i]) * 128 for i in range(NCH)]   # [2176, 2048, 2048]
TBL = [NCORES * s for s in CSZ]                 # AG table rows (int16-safe)


def _install_hooks():
    """antenv.axon_hooks shim so trace=True works in this image (optional)."""
    try:
        import antenv
        if "antenv.axon_hooks" not in sys.modules:
            mod = types.ModuleType("antenv.axon_hooks")
            mod._hook = None
            mod.set_axon_ntff_profile_hook = lambda h: setattr(mod, "_hook", h)
            mod.get_axon_ntff_profile_hook = lambda: mod._hook
            sys.modules["antenv.axon_hooks"] = mod
            antenv.axon_hooks = mod
        from antenv.axon_hooks import get_axon_ntff_profile_hook, set_axon_ntff_profile_hook
        if get_axon_ntff_profile_hook() is None:
            from trn_agent_boot.trn_boot import _ntff_profile_via_ctypes
            set_axon_ntff_profile_hook(_ntff_profile_via_ctypes("/opt/axon/libaxon_pjrt.so"))
        import concourse.bass_utils as bu
        bu.upload_artifacts = lambda tmpdir: f"file://{tmpdir}"
    except Exception:
        pass


def _preprocess(edge_index):
    """Edge lists per core, grouped by (dst block, src chunk), padded per-tile."""
    src = np.asarray(edge_index[0], np.int64)
    dst = np.asarray(edge_index[1], np.int64)
    core = dst // S
    dl = (dst % S).astype(np.int64)
    blk = dl // 128
    col = dl % 128
    sloc = src % S
    chunk = np.digitize(sloc, [CST[1], CST[2]])
    cst = np.asarray(CST)[chunk]
    csz = np.asarray(CSZ)[chunk]
    grow = (src // S) * csz + (sloc - cst)   # row within its chunk-table

    deg = np.bincount(core * S + dl, minlength=N).reshape(NCORES, S)

    order = np.lexsort((grow, blk, chunk, core))
    core_s, ch_s, blk_s, col_s, row_s = (core[order], chunk[order], blk[order],
                                         col[order], grow[order])

    key = (core_s * NCH + ch_s) * NBLK + blk_s
    counts = np.bincount(key, minlength=NCORES * NCH * NBLK).reshape(NCORES, NCH, NBLK)
    tiles_hb = np.ceil(counts.max(axis=0) / 128).astype(np.int64)   # [NCH, NBLK]
    tiles_hb = np.maximum(tiles_hb, 1)

    pad_hb = tiles_hb * 128
    tot_h = pad_hb.sum(axis=1)
    seg_off = np.zeros((NCH, NBLK), np.int64)
    seg_off[:, 1:] = np.cumsum(pad_hb, axis=1)[:, :-1]

    srcpad = np.zeros((NCORES, NCH), dtype=object)
    colpad = np.zeros((NCORES, NCH), dtype=object)
    for c in range(NCORES):
        for h in range(NCH):
            srcpad[c, h] = np.zeros(int(tot_h[h]), np.int64)
            colpad[c, h] = np.full(int(tot_h[h]), -1, np.int64)
    grp = key
    first = np.r_[True, grp[1:] != grp[:-1]]
    gidx = np.arange(len(grp)) - np.maximum.accumulate(np.where(first, np.arange(len(grp)), 0))
    pos = seg_off[ch_s, blk_s] + gidx
    for c in range(NCORES):
        m = core_s == c
        for h in range(NCH):
            mh = m & (ch_s == h)
            p = pos[mh]
            srcpad[c, h][p] = row_s[mh]
            colpad[c, h][p] = col_s[mh]

    return {
        "tiles_hb": tiles_hb, "seg_off": seg_off,
        "srcpad": srcpad, "colpad": colpad, "deg": deg,
    }


def _build_callplan(tiles_hb):
    """Gather call plan (compile-time, same for every core)."""
    calls = []
    block_calls = {b: {h: [] for h in range(NCH)} for b in range(NBLK)}
    block_tiles = {b: [] for b in range(NBLK)}
    tile_ctr = [0] * NCH
    ht_base = np.concatenate([[0], np.cumsum(tiles_hb.sum(axis=1))])[:NCH]
    for b in range(NBLK):
        for h in range(NCH):
            nt = int(tiles_hb[h, b])
            done = 0
            while done < nt:
                k = min(TPC, nt - done)
                ci = len(calls)
                calls.append(dict(h=h, k=k, tile_base=tile_ctr[h], blk=b, q=0))
                block_calls[b][h].append(ci)
                for j in range(k):
                    dcol = int(ht_base[h]) + tile_ctr[h] + j
                    block_tiles[b].append((ci, j, dcol))
                tile_ctr[h] += k
                done += k
    qload = [0, 0, 0, 0]
    for cl in calls:
        q = qload.index(min(qload))
        cl["q"] = q
        qload[q] += cl["k"]
    return calls, block_calls, block_tiles


def _blk_oh_ranges(tiles_hb):
    """Per block, per chunk: (first dcol, ntiles) of its one-hot image range."""
    ht_base = np.concatenate([[0], np.cumsum(tiles_hb.sum(axis=1))])[:NCH]
    out = []
    for b in range(NBLK):
        r = []
        for h in range(NCH):
            start = int(ht_base[h]) + int(tiles_hb[h, :b].sum())
            r.append((start, int(tiles_hb[h, b])))
        out.append(r)
    return out


def _idx_arrays(pre, calls, core):
    """int16 idx image [128, ncalls*CPC] and one-hot image [128, tiles*128] f8."""
    import ml_dtypes as _ml
    ncalls = len(calls)
    idx_img = np.zeros((16, ncalls * CPC), np.int16)
    tiles_total = int(pre["tiles_hb"].sum())
    dstloc = np.full((128, tiles_total), -1, np.int64)
    ht_base = np.concatenate([[0], np.cumsum(pre["tiles_hb"].sum(axis=1))])[:NCH]
    for ci, cl in enumerate(calls):
        h, k, tb, b = cl["h"], cl["k"], cl["tile_base"], cl["blk"]
        e0 = int(pre["seg_off"][h, b]) + (tb - int(pre["tiles_hb"][h, :b].sum())) * 128
        nidx = k * 128
        seg_src = pre["srcpad"][core, h][e0:e0 + nidx]
        seg_col = pre["colpad"][core, h][e0:e0 + nidx]
        idx_img[:, ci * CPC: ci * CPC + (nidx // 16)] = seg_src.reshape(-1, 16).T.astype(np.int16)
        for t in range(k):
            dcol = int(ht_base[h]) + tb + t
            dstloc[:, dcol] = seg_col[t * 128:(t + 1) * 128]
    return np.tile(idx_img, (8, 1)), dstloc.astype(np.float32).astype(_ml.bfloat16)


def _build_bass(pre, calls, block_calls, block_tiles, ncalls_cols, tiles_total):
    import concourse.bass as bass
    import concourse.bacc as bacc
    import concourse.mybir as mybir
    import concourse.tile as tile

    FP32 = mybir.dt.float32
    BF16 = mybir.dt.bfloat16
    F8 = mybir.dt.float8e4
    I16 = mybir.dt.int16
    AL = mybir.AluOpType
    AF = mybir.ActivationFunctionType

    blk_oh = _blk_oh_ranges(pre["tiles_hb"])

    nc = bacc.Bacc("TRN2", target_bir_lowering=False, debug=False,
                   enable_asserts=False, num_devices=NCORES, num_swdge_queues=4)

    x0T = nc.dram_tensor("x0T", [128, SP], BF16, kind="ExternalInput")
    x1T = nc.dram_tensor("x1T", [128, SP], BF16, kind="ExternalInput")
    x0dT = nc.dram_tensor("x0dT", [128, SP], BF16, kind="ExternalInput")
    x1dT = nc.dram_tensor("x1dT", [128, SP], BF16, kind="ExternalInput")
    wl0 = nc.dram_tensor("wl0", [128, 128], BF16, kind="ExternalInput")
    wr0 = nc.dram_tensor("wr0", [128, 128], BF16, kind="ExternalInput")
    wl1 = nc.dram_tensor("wl1", [128, 128], BF16, kind="ExternalInput")
    wr1 = nc.dram_tensor("wr1", [128, 128], BF16, kind="ExternalInput")
    wlm = nc.dram_tensor("wlm", [256, 256], BF16, kind="ExternalInput")
    wrm = nc.dram_tensor("wrm", [256, 256], BF16, kind="ExternalInput")
    wlo = nc.dram_tensor("wlo", [256, 64], BF16, kind="ExternalInput")
    wro = nc.dram_tensor("wro", [256, 64], BF16, kind="ExternalInput")
    b01d = nc.dram_tensor("b01", [1, 256], BF16, kind="ExternalInput")
    bmd = nc.dram_tensor("bm", [1, 256], BF16, kind="ExternalInput")
    bod = nc.dram_tensor("bo", [1, 64], BF16, kind="ExternalInput")
    idxd = nc.dram_tensor("idx", [128, ncalls_cols], I16, kind="ExternalInput")
    dstld = nc.dram_tensor("dstl", [128, tiles_total], BF16, kind="ExternalInput")
    invrd = nc.dram_tensor("invr", [128, SP], BF16, kind="ExternalInput")
    degrd = nc.dram_tensor("degr", [128, SP], BF16, kind="ExternalInput")
    invcd = nc.dram_tensor("invc", [128, NBLK], FP32, kind="ExternalInput")
    outd = nc.dram_tensor("out", [S, DO], FP32, kind="ExternalOutput")

    with tile.TileContext(nc) as tc:
        with (
            tc.tile_pool(name="const", bufs=1) as cp,
            tc.tile_pool(name="acts", bufs=1) as hp,
            tc.tile_pool(name="g", bufs=28) as gp,
            tc.tile_pool(name="oh", bufs=6) as ohp,
            tc.tile_pool(name="xs", bufs=6) as xsp,
            # PSUM budget (8 banks): ps0/ps1 (3 bufs each = 6 banks) + py (2)
            tc.tile_pool(name="ps", bufs=3, space="PSUM") as psp,
            tc.tile_pool(name="psy", bufs=2, space="PSUM") as psyp,
            tc.tile_pool(name="ev", bufs=4) as evp,
            tc.tile_pool(name="dram", bufs=1, space="DRAM") as dp,
        ):
            def load(name, dt_, shape, src):
                t = cp.tile(shape, dt_, name=name)
                nc.sync.dma_start(out=t[:], in_=src)
                return t

            wl0t = load("wl0t", BF16, [128, 128], wl0[:])
            wr0t = load("wr0t", BF16, [128, 128], wr0[:])
            wl1t = load("wl1t", BF16, [128, 128], wl1[:])
            wr1t = load("wr1t", BF16, [128, 128], wr1[:])
            wlmt = [load(f"wlmt{i}", BF16, [128, 256], wlm[i * 128:(i + 1) * 128, :]) for i in range(2)]
            wrmt = [load(f"wrmt{i}", BF16, [128, 256], wrm[i * 128:(i + 1) * 128, :]) for i in range(2)]
            wlot = [load(f"wlot{i}", BF16, [128, 64], wlo[i * 128:(i + 1) * 128, :]) for i in range(2)]
            wrot = [load(f"wrot{i}", BF16, [128, 64], wro[i * 128:(i + 1) * 128, :]) for i in range(2)]
            b01t = load("b01t", BF16, [1, 256], b01d[:])
            bmt = load("bmt", BF16, [1, 256], bmd[:])
            bot = load("bot", BF16, [1, 64], bod[:])
            idxt = load("idxt", I16, [128, ncalls_cols], idxd[:])
            invr = load("invrt", BF16, [128, SP], invrd[:])
            degr = load("degrt", BF16, [128, SP], degrd[:])
            invc = load("invct", FP32, [128, NBLK], invcd[:])
            dstl = load("dstlt", BF16, [128, tiles_total], dstld[:])

            iota_i = cp.tile([128, MT, 128], mybir.dt.int32, name="iota_i")
            nc.gpsimd.iota(iota_i[:], pattern=[[0, MT], [1, 128]], base=0,
                           channel_multiplier=0)
            iota_bf = cp.tile([128, MT, 128], BF16, name="iota_bf")
            nc.vector.tensor_copy(out=iota_bf[:], in_=iota_i[:])

            # memset gather pool once: padded idx slots gather row 0 (finite),
            # keeping every slot's stale data finite for zero one-hot columns.
            for i in range(28):
                gz = gp.tile([128, TPC, D1], F8, name="gz", tag="g")
                nc.vector.memset(gz[:], 0.0)

            warm_own = dp.tile([8, 256], F8, name="warm_own")
            warm_tab = dp.tile([64, 256], F8, name="warm_tab",
                               addr_space="Shared" if NCORES > 4 else "Local")
            wz = evp.tile([8, 256], F8, name="wz", tag="wz")
            nc.vector.memset(wz[:], 0.0)
            nc.sync.dma_start(out=warm_own[:], in_=wz[:])
            nc.gpsimd.collective_compute(
                "AllGather", AL.bypass, replica_groups=[list(range(NCORES))],
                ins=[warm_own[:]], outs=[warm_tab[:]])

            hT = [hp.tile([128, SP], BF16, name=f"hT{i}") for i in range(2)]
            h2T = [hp.tile([128, SP], BF16, name=f"h2T{i}") for i in range(2)]

            shared = "Shared" if NCORES > 4 else "Local"

            def mk_tables(name, width):
                own = [dp.tile([CSZ[h], width], F8, name=f"{name}_own{h}")
                       for h in range(NCH)]
                tab = [dp.tile([TBL[h], width], F8, name=f"{name}{h}",
                               addr_space=shared) for h in range(NCH)]
                return own, tab

            y01_own, Y01 = mk_tables("y01", D1)
            ym_own, Ym = mk_tables("ym", DM)
            yo_own, Yo = mk_tables("yo", 256)

            def chunk_of_block(b):
                return 0 if b < CBLK[1] else (1 if b < CBLK[2] else 2)

            def write_y(dsts, b, src_tile, dcols):
                h = chunk_of_block(b)
                r0 = b * 128 - CST[h]
                nc.sync.dma_start(out=dsts[h][r0:r0 + 128, 0:dcols],
                                  in_=src_tile[:, 0:dcols])

            RG = [list(range(NCORES))]

            def blk_sl(b):
                return slice(b * 128, (b + 1) * 128)

            def make_ags(own, tab):
                def mk(h):
                    def f():
                        nc.gpsimd.collective_compute(
                            "AllGather", AL.bypass, replica_groups=RG,
                            ins=[own[h][:]], outs=[tab[h][:]])
                    return f
                return [mk(h) for h in range(NCH)]

            def load_oh(b):
                """Build this block's one-hot tiles with a single DVE IS_EQ."""
                tiles = {}
                for h in range(NCH):
                    start, nt = blk_oh[b][h]
                    t = ohp.tile([128, MT, 128], F8, name=f"ohb{h}", tag=f"oh{h}")
                    nc.vector.tensor_tensor(
                        out=t[:, 0:nt, :], in0=iota_bf[:, 0:nt, :],
                        in1=dstl[:, start:start + nt].to_broadcast([128, nt, 128]),
                        op=AL.is_equal)
                    tiles[h] = (t, start)
                return tiles

            # AG kick: chunk i kicks a few blocks after its rows are written
            # so the kick's input-wait is already satisfied; tail at loop end.
            ag_at = {CBLK[1] + 3: 0, CBLK[2] + 3: 1, CBLK[3] - 1: 2}

            # ================= L1 pre: y01_own = [x0@Wl0 | x1@Wl1] =========
            ags01 = make_ags(y01_own, Y01)
            ag_at_pre = {CBLK[1] - 1: 0, CBLK[2] - 1: 1, CBLK[3] - 1: 2}
            for b in range(NBLK):
                x0b = xsp.tile([128, 128], BF16, name="x0b", tag="x0b")
                nc.sync.dma_start(out=x0b[:], in_=x0T[:, blk_sl(b)])
                x1b = xsp.tile([128, 128], BF16, name="x1b", tag="x1b")
                nc.sync.dma_start(out=x1b[:], in_=x1T[:, blk_sl(b)])
                py0 = psp.tile([128, 128], FP32, name="py0", tag="ps0")
                py1 = psp.tile([128, 128], FP32, name="py1", tag="ps1")
                nc.tensor.matmul(py0[:], lhsT=x0b[:], rhs=wl0t[:], start=True, stop=True)
                nc.tensor.matmul(py1[:], lhsT=x1b[:], rhs=wl1t[:], start=True, stop=True)
                evy = evp.tile([128, 256], F8, name="evy", tag="evy", padded_shape=[128, 512])
                nc.vector.tensor_copy(out=evy[:, 0:128], in_=py0[:])
                nc.vector.tensor_copy(out=evy[:, 128:256], in_=py1[:])
                write_y(y01_own, b, evy, D1)
                if b in ag_at_pre:
                    ags01[ag_at_pre[b]]()

            # ================= aggregation layer (L1/L2) =====================
            def agg_layer(Ytab, wr_tiles, bias_t, h_src, h_dst, wl_next, y_next,
                          d_next, ags_next):
                gtiles = {}
                qn = [0]

                def emit_gathers(cis):
                    for ci in cis:
                        cl = calls[ci]
                        k = cl["k"]
                        g = gp.tile([128, TPC, D1], F8, name="g", tag="g")
                        nc.gpsimd.dma_gather(
                            out_ap=g[:, 0:k, :],
                            in_ap=Ytab[cl["h"]][:],
                            idxs_ap=idxt[:, ci * CPC: ci * CPC + (k * 128) // 16],
                            num_idxs=k * 128, num_idxs_reg=k * 128,
                            elem_size=D1, queue_num=cl["q"])
                        gtiles[ci] = g

                W = 7
                for b in range(W):
                    emit_gathers(block_calls[b][0])
                    emit_gathers(block_calls[b][1])
                for b in range(NBLK):
                    emit_gathers(block_calls[b][2])
                    if b + W < NBLK:
                        emit_gathers(block_calls[b + W][0])
                        emit_gathers(block_calls[b + W][1])
                    ohb = load_oh(b)
                    ps0 = psp.tile([128, 128], FP32, name="ps0", tag="ps0")
                    ps1 = psp.tile([128, 128], FP32, name="ps1", tag="ps1")
                    if h_src is None:
                        x0b = xsp.tile([128, 128], BF16, name="x0b2", tag="xd0")
                        nc.sync.dma_start(out=x0b[:], in_=x0dT[:, blk_sl(b)])
                        x1b = xsp.tile([128, 128], BF16, name="x1b2", tag="xd1")
                        nc.sync.dma_start(out=x1b[:], in_=x1dT[:, blk_sl(b)])
                        nc.tensor.matmul(ps0[:], lhsT=wr0t[:], rhs=x0b[:], start=True, stop=False)
                        nc.tensor.matmul(ps1[:], lhsT=wr1t[:], rhs=x1b[:], start=True, stop=False)
                    else:
                        hd0 = evp.tile([128, 128], BF16, name="hd0", tag="hd0")
                        nc.vector.tensor_tensor(out=hd0[:], in0=h_src[0][:, blk_sl(b)],
                                                in1=degr[:, blk_sl(b)], op=AL.mult)
                        hd1 = evp.tile([128, 128], BF16, name="hd1", tag="hd1")
                        nc.vector.tensor_tensor(out=hd1[:], in0=h_src[1][:, blk_sl(b)],
                                                in1=degr[:, blk_sl(b)], op=AL.mult)
                        nc.tensor.matmul(ps0[:], lhsT=wr_tiles[0][:, 0:128], rhs=hd0[:], start=True, stop=False)
                        nc.tensor.matmul(ps0[:], lhsT=wr_tiles[1][:, 0:128], rhs=hd1[:], start=False, stop=False)
                        nc.tensor.matmul(ps1[:], lhsT=wr_tiles[0][:, 128:256], rhs=hd0[:], start=True, stop=False)
                        nc.tensor.matmul(ps1[:], lhsT=wr_tiles[1][:, 128:256], rhs=hd1[:], start=False, stop=False)
                    nc.tensor.matmul(ps0[:], lhsT=bias_t[0:1, 0:128], rhs=degr[0:1, blk_sl(b)],
                                     start=False, stop=False)
                    nc.tensor.matmul(ps1[:], lhsT=bias_t[0:1, 128:256], rhs=degr[0:1, blk_sl(b)],
                                     start=False, stop=False)
                    tl = block_tiles[b]
                    for n, (ci, slot, dcol) in enumerate(tl):
                        g = gtiles[ci]
                        oht, start = ohb[calls[ci]["h"]]
                        j = dcol - start
                        last = (n == len(tl) - 1)
                        nc.tensor.matmul(ps0[:], lhsT=g[:, slot, 0:128], rhs=oht[:, j, :],
                                         start=False, stop=last)
                        nc.tensor.matmul(ps1[:], lhsT=g[:, slot, 128:256], rhs=oht[:, j, :],
                                         start=False, stop=last)
                    # epilogue: h = relu(ps) * invd  (relu commutes with the
                    # positive per-column scale)
                    rt0 = evp.tile([128, 128], BF16, name="rt0", tag="rt0")
                    nc.scalar.activation(rt0[:], ps0[:], AF.Relu)
                    nc.vector.tensor_tensor(out=h_dst[0][:, blk_sl(b)], in0=rt0[:],
                                            in1=invr[:, blk_sl(b)], op=AL.mult)
                    rt1 = evp.tile([128, 128], BF16, name="rt1", tag="rt1")
                    nc.scalar.activation(rt1[:], ps1[:], AF.Relu)
                    nc.vector.tensor_tensor(out=h_dst[1][:, blk_sl(b)], in0=rt1[:],
                                            in1=invr[:, blk_sl(b)], op=AL.mult)
                    pyn = psyp.tile([128, d_next], FP32, name="pyn", tag="py",
                                    padded_shape=[128, 256])
                    nc.tensor.matmul(pyn[:], lhsT=h_dst[0][:, blk_sl(b)], rhs=wl_next[0][:],
                                     start=True, stop=False)
                    nc.tensor.matmul(pyn[:], lhsT=h_dst[1][:, blk_sl(b)], rhs=wl_next[1][:],
                                     start=False, stop=True)
                    evn = evp.tile([128, d_next], F8, name="evn", tag="evy",
                                   padded_shape=[128, 512])
                    nc.vector.tensor_copy(out=evn[:], in_=pyn[:])
                    write_y(y_next, b, evn, d_next)
                    if b in ag_at:
                        ags_next[ag_at[b]]()

            agg_layer(Y01, None, b01t, None, hT, wlmt, ym_own, DM,
                      make_ags(ym_own, Ym))
            agg_layer(Ym, wrmt, bmt, hT, h2T, wlot, yo_own, DO,
                      make_ags(yo_own, Yo))

            # ================= L3: out[node, 64] ============================
            qn3 = [0]
            gtiles3 = {}

            def emit_gathers3(cis):
                for ci in cis:
                    cl = calls[ci]
                    k = cl["k"]
                    g3 = gp.tile([128, TPC, 256], F8, name="g3", tag="g")
                    nc.gpsimd.dma_gather(
                        out_ap=g3[:, 0:k, :], in_ap=Yo[cl["h"]][:],
                        idxs_ap=idxt[:, ci * CPC: ci * CPC + (k * 128) // 16],
                        num_idxs=k * 128, num_idxs_reg=k * 128,
                        elem_size=256, queue_num=cl["q"])
                    gtiles3[ci] = g3

            W3 = 7
            for b in range(W3):
                emit_gathers3(block_calls[b][0])
                emit_gathers3(block_calls[b][1])
            for b in range(NBLK):
                emit_gathers3(block_calls[b][2])
                if b + W3 < NBLK:
                    emit_gathers3(block_calls[b + W3][0])
                    emit_gathers3(block_calls[b + W3][1])
                ohb = load_oh(b)
                ps3 = psp.tile([128, DO], FP32, name="ps3", tag="ps0",
                               padded_shape=[128, 128])
                hd0 = evp.tile([128, 128], BF16, name="hd20", tag="hd0")
                nc.vector.tensor_tensor(out=hd0[:], in0=h2T[0][:, blk_sl(b)],
                                        in1=degr[:, blk_sl(b)], op=AL.mult)
                hd1 = evp.tile([128, 128], BF16, name="hd21", tag="hd1")
                nc.vector.tensor_tensor(out=hd1[:], in0=h2T[1][:, blk_sl(b)],
                                        in1=degr[:, blk_sl(b)], op=AL.mult)
                nc.tensor.matmul(ps3[:], lhsT=hd0[:], rhs=wrot[0][:],
                                 start=True, stop=False)
                nc.tensor.matmul(ps3[:], lhsT=hd1[:], rhs=wrot[1][:],
                                 start=False, stop=False)
                nc.tensor.matmul(ps3[:], lhsT=degr[0:1, blk_sl(b)], rhs=bot[0:1, :],
                                 start=False, stop=False)
                tl = block_tiles[b]
                for n, (ci, slot, dcol) in enumerate(tl):
                    g3 = gtiles3[ci]
                    oht, start = ohb[calls[ci]["h"]]
                    j = dcol - start
                    nc.tensor.matmul(ps3[:], lhsT=oht[:, j, :], rhs=g3[:, slot, 0:64],
                                     start=False, stop=(n == len(tl) - 1))
                osb = evp.tile([128, DO], FP32, name="osb", tag="osb")
                nc.scalar.activation(osb[:], ps3[:], AF.Copy,
                                     scale=invc[:, b:b + 1])
                rows = min(128, S - b * 128)
                nc.sync.dma_start(out=outd[b * 128: b * 128 + rows, :],
                                  in_=osb[0:rows, :])

    nc.finalize()
    return nc


_CACHE = {}


def _make_inmaps(inputs, pre, calls):
    import ml_dtypes as _ml
    BF = _ml.bfloat16
    x0 = np.asarray(inputs["x0"], np.float32)
    x1 = np.asarray(inputs["x1"], np.float32)
    deg = pre["deg"]
    bf16 = lambda a: np.ascontiguousarray(a).astype(BF)
    in_maps = []
    for c in range(NCORES):
        degc = np.maximum(deg[c], 1.0).astype(np.float32)
        invd = (1.0 / degc).astype(np.float32)
        degc_p = np.ones(SP, np.float32)
        degc_p[:S] = degc
        invd_p = np.ones(SP, np.float32)
        invd_p[:S] = invd
        idx_img, dstloc = _idx_arrays(pre, calls, c)
        x0c = np.zeros((128, SP), np.float32)
        x0c[:, :S] = x0[c * S:(c + 1) * S, :].T
        x1c = np.zeros((128, SP), np.float32)
        x1c[:, :S] = x1[c * S:(c + 1) * S, :].T
        x0dc = x0c * degc_p[None, :]
        x1dc = x1c * degc_p[None, :]
        in_maps.append({
            "x0T": bf16(x0c), "x1T": bf16(x1c),
            "x0dT": bf16(x0dc), "x1dT": bf16(x1dc),
            "wl0": bf16(inputs["Wl0"]), "wr0": bf16(inputs["Wr0"]),
            "wl1": bf16(inputs["Wl1"]), "wr1": bf16(inputs["Wr1"]),
            "wlm": bf16(inputs["Wlm"]), "wrm": bf16(inputs["Wrm"]),
            "wlo": bf16(inputs["Wlo"]), "wro": bf16(inputs["Wro"]),
            "b01": bf16(np.concatenate([np.asarray(inputs["b0"], np.float32),
                                        np.asarray(inputs["b1"], np.float32)])[None, :]),
            "bm": bf16(np.asarray(inputs["bm"], np.float32)[None, :]),
            "bo": bf16(np.asarray(inputs["bo"], np.float32)[None, :]),
            "idx": idx_img, "dstl": dstloc,
            "invr": bf16(np.broadcast_to(invd_p[None, :], (128, SP))),
            "degr": bf16(np.broadcast_to(degc_p[None, :], (128, SP))),
            "invc": np.ascontiguousarray(invd_p[:NBLK * 128].reshape(NBLK, 128).T,
                                         np.float32),
        })
    return in_maps


def _get_program(edge_index):
    if "prog" in _CACHE:
        return _CACHE["prog"]
    pre = _preprocess(edge_index)
    calls, block_calls, block_tiles = _build_callplan(pre["tiles_hb"])
    tiles_total = int(pre["tiles_hb"].sum())
    nc = _build_bass(pre, calls, block_calls, block_tiles, len(calls) * CPC, tiles_total)
    _CACHE["prog"] = (nc, pre, calls)
    return _CACHE["prog"]


LAST_EXEC_NS = None


def kernel(**inputs):
    global LAST_EXEC_NS
    _install_hooks()
    from concourse.bass_utils import run_bass_kernel_spmd

    nc, pre, calls = _get_program(inputs["edge_index"])
    in_maps = _make_inmaps(inputs, pre, calls)
    trace = os.environ.get("KERNEL_TRACE", "0") == "1"
    res = run_bass_kernel_spmd(nc, in_maps, list(range(NCORES)), trace=trace)
    LAST_EXEC_NS = res.exec_time_ns
    return np.concatenate([np.asarray(res.results[c]["out"]) for c in range(NCORES)], axis=0)


# revision 40
# speedup vs baseline: 1.0106x; 1.0023x over previous
"""Self-contained Trainium2 Bass kernel for 4-layer GraphSAGE (nn_LASAGE).

Strategy (v4 — fp8 tables, host one-hots, 3-chunk pipelined AllGathers):
  - Nodes dst-sharded across 8 cores (6250/core, padded to 6272 = 49 blocks of 128).
  - Aggregation is done POST-matmul: agg(x)@Wl == agg(x@Wl), so per layer each
    core computes y = h @ Wl for its own shard; the full Y table [50176, d] is
    replicated via THREE chunked AllGathers (blocks [0:17) [17:33) [33:49)),
    kicked as soon as each chunk's rows are written. Chunk tables stay under
    the int16 idx limit (17408 rows). Edges gather y[src] rows with dma_gather
    (fp8e4, 256B rows) on 4 SWDGE queues.
  - Chunk-0/1 gather calls of the first W blocks are PREFLIGHTED at layer
    start: their AGs completed mid-previous-layer, so they fill the DMA
    engines while the previous layer's tail chunk-2 AllGather (only 10/49 of
    a table) is still landing. Gather calls are capped at 768 idxs: >768
    crashes or falls off a ucode performance cliff (1024 runs 50x slower).
  - Scatter-add into dst blocks via one-hot matmuls on the PE. The one-hot is
    UNSCALED {0,1} fp8, built per block with a single DVE IS_EQ against a
    host-provided bf16 dst-column map. Mean-normalization moves to the edges:
      out = invd[dst] * (gather_sum + degc[dst]*(x@Wr) + degc[dst]*b)
    with degc = max(deg,1) pre-scaled Wr inputs and an invd epilogue
    (ACT relu + DVE column-scale for L1/L2's transposed psum; a single ACT
    Copy with per-partition scale for L3's [dst, feat] psum).
  - All dense operands (x, weights, h storage) are bf16; psum stays fp32.
  - Layer1 fuses conv0+conv1 (concat -> 256 feat). Layer3 (output, d=64) uses
    non-transposed psum (lhsT=onehot) so rows DMA straight to the output;
    its fp8 table rows are 256-wide with only cols 0:64 valid.
"""
import sys, os, types

sys.path.insert(0, "/opt/trn_rl_repo")
import numpy as np

N = 50000
E = 800000
NCORES = 8
S = N // NCORES            # 6250 real nodes per core
SP = 6272                  # padded (49 blocks of 128)
NBLK = SP // 128
D1 = 256                   # concat(h0, h1)
DM = 256
DO = 64
MAXI = 768                 # max idxs per dma_gather call
TPC = MAXI // 128          # tiles per full call (6)
CPC = MAXI // 16           # idx-image cols per call (48)
MT = 9                     # max tiles per (chunk, dst-block) segment
NCH = 3
CBLK = [0, 22, 39, 49]     # chunk boundaries in blocks (small tail AG)
CST = [b * 128 for b in CBLK[:-1]]              # chunk start rows (per core)
CSZ = [(CBLK[i + 1] - CBLK[i]) * 128 for i in range(NCH)]   # [2176, 2048, 2048]
TBL = [NCORES * s for s in CSZ]                 # AG table rows (int16-safe)


def _install_hooks():
    """antenv.axon_hooks shim so trace=True works in this image (optional)."""
    try:
        import antenv
        if "antenv.axon_hooks" not in sys.modules:
            mod = types.ModuleType("antenv.axon_hooks")
            mod._hook = None
            mod.set_axon_ntff_profile_hook = lambda h: setattr(mod, "_hook", h)
            mod.get_axon_ntff_profile_hook = lambda: mod._hook
            sys.modules["antenv.axon_hooks"] = mod
            antenv.axon_hooks = mod
        from antenv.axon_hooks import get_axon_ntff_profile_hook, set_axon_ntff_profile_hook
        if get_axon_ntff_profile_hook() is None:
            from trn_agent_boot.trn_boot import _ntff_profile_via_ctypes
            set_axon_ntff_profile_hook(_ntff_profile_via_ctypes("/opt/axon/libaxon_pjrt.so"))
        import concourse.bass_utils as bu
        bu.upload_artifacts = lambda tmpdir: f"file://{tmpdir}"
    except Exception:
        pass


def _preprocess(edge_index):
    """Edge lists per core, grouped by (dst block, src chunk), padded per-tile."""
    src = np.asarray(edge_index[0], np.int64)
    dst = np.asarray(edge_index[1], np.int64)
    core = dst // S
    dl = (dst % S).astype(np.int64)
    blk = dl // 128
    col = dl % 128
    sloc = src % S
    chunk = np.digitize(sloc, [CST[1], CST[2]])
    cst = np.asarray(CST)[chunk]
    csz = np.asarray(CSZ)[chunk]
    grow = (src // S) * csz + (sloc - cst)   # row within its chunk-table

    deg = np.bincount(core * S + dl, minlength=N).reshape(NCORES, S)

    order = np.lexsort((grow, blk, chunk, core))
    core_s, ch_s, blk_s, col_s, row_s = (core[order], chunk[order], blk[order],
                                         col[order], grow[order])

    key = (core_s * NCH + ch_s) * NBLK + blk_s
    counts = np.bincount(key, minlength=NCORES * NCH * NBLK).reshape(NCORES, NCH, NBLK)
    tiles_hb = np.ceil(counts.max(axis=0) / 128).astype(np.int64)   # [NCH, NBLK]
    tiles_hb = np.maximum(tiles_hb, 1)

    pad_hb = tiles_hb * 128
    tot_h = pad_hb.sum(axis=1)
    seg_off = np.zeros((NCH, NBLK), np.int64)
    seg_off[:, 1:] = np.cumsum(pad_hb, axis=1)[:, :-1]

    srcpad = np.zeros((NCORES, NCH), dtype=object)
    colpad = np.zeros((NCORES, NCH), dtype=object)
    for c in range(NCORES):
        for h in range(NCH):
            srcpad[c, h] = np.zeros(int(tot_h[h]), np.int64)
            colpad[c, h] = np.full(int(tot_h[h]), -1, np.int64)
    grp = key
    first = np.r_[True, grp[1:] != grp[:-1]]
    gidx = np.arange(len(grp)) - np.maximum.accumulate(np.where(first, np.arange(len(grp)), 0))
    pos = seg_off[ch_s, blk_s] + gidx
    for c in range(NCORES):
        m = core_s == c
        for h in range(NCH):
            mh = m & (ch_s == h)
            p = pos[mh]
            srcpad[c, h][p] = row_s[mh]
            colpad[c, h][p] = col_s[mh]

    return {
        "tiles_hb": tiles_hb, "seg_off": seg_off,
        "srcpad": srcpad, "colpad": colpad, "deg": deg,
    }


def _build_callplan(tiles_hb):
    """Gather call plan (compile-time, same for every core)."""
    calls = []
    block_calls = {b: {h: [] for h in range(NCH)} for b in range(NBLK)}
    block_tiles = {b: [] for b in range(NBLK)}
    tile_ctr = [0] * NCH
    ht_base = np.concatenate([[0], np.cumsum(tiles_hb.sum(axis=1))])[:NCH]
    for b in range(NBLK):
        for h in range(NCH):
            nt = int(tiles_hb[h, b])
            done = 0
            while done < nt:
                k = min(TPC, nt - done)
                ci = len(calls)
                calls.append(dict(h=h, k=k, tile_base=tile_ctr[h], blk=b, q=0))
                block_calls[b][h].append(ci)
                for j in range(k):
                    dcol = int(ht_base[h]) + tile_ctr[h] + j
                    block_tiles[b].append((ci, j, dcol))
                tile_ctr[h] += k
                done += k
    qload = [0, 0, 0, 0]
    for cl in calls:
        q = qload.index(min(qload))
        cl["q"] = q
        qload[q] += cl["k"]
    return calls, block_calls, block_tiles


def _blk_oh_ranges(tiles_hb):
    """Per block, per chunk: (first dcol, ntiles) of its one-hot image range."""
    ht_base = np.concatenate([[0], np.cumsum(tiles_hb.sum(axis=1))])[:NCH]
    out = []
    for b in range(NBLK):
        r = []
        for h in range(NCH):
            start = int(ht_base[h]) + int(tiles_hb[h, :b].sum())
            r.append((start, int(tiles_hb[h, b])))
        out.append(r)
    return out


def _idx_arrays(pre, calls, core):
    """int16 idx image [128, ncalls*CPC] and one-hot image [128, tiles*128] f8."""
    import ml_dtypes as _ml
    ncalls = len(calls)
    idx_img = np.zeros((16, ncalls * CPC), np.int16)
    tiles_total = int(pre["tiles_hb"].sum())
    dstloc = np.full((128, tiles_total), -1, np.int64)
    ht_base = np.concatenate([[0], np.cumsum(pre["tiles_hb"].sum(axis=1))])[:NCH]
    for ci, cl in enumerate(calls):
        h, k, tb, b = cl["h"], cl["k"], cl["tile_base"], cl["blk"]
        e0 = int(pre["seg_off"][h, b]) + (tb - int(pre["tiles_hb"][h, :b].sum())) * 128
        nidx = k * 128
        seg_src = pre["srcpad"][core, h][e0:e0 + nidx]
        seg_col = pre["colpad"][core, h][e0:e0 + nidx]
        idx_img[:, ci * CPC: ci * CPC + (nidx // 16)] = seg_src.reshape(-1, 16).T.astype(np.int16)
        for t in range(k):
            dcol = int(ht_base[h]) + tb + t
            dstloc[:, dcol] = seg_col[t * 128:(t + 1) * 128]
    return np.tile(idx_img, (8, 1)), dstloc.astype(np.float32).astype(_ml.bfloat16)


def _build_bass(pre, calls, block_calls, block_tiles, ncalls_cols, tiles_total):
    import concourse.bass as bass
    import concourse.bacc as bacc
    import concourse.mybir as mybir
    import concourse.tile as tile

    FP32 = mybir.dt.float32
    BF16 = mybir.dt.bfloat16
    F8 = mybir.dt.float8e4
    I16 = mybir.dt.int16
    AL = mybir.AluOpType
    AF = mybir.ActivationFunctionType

    blk_oh = _blk_oh_ranges(pre["tiles_hb"])

    nc = bacc.Bacc("TRN2", target_bir_lowering=False, debug=False,
                   enable_asserts=False, num_devices=NCORES, num_swdge_queues=4)

    x0T = nc.dram_tensor("x0T", [128, SP], BF16, kind="ExternalInput")
    x1T = nc.dram_tensor("x1T", [128, SP], BF16, kind="ExternalInput")
    x0dT = nc.dram_tensor("x0dT", [128, SP], BF16, kind="ExternalInput")
    x1dT = nc.dram_tensor("x1dT", [128, SP], BF16, kind="ExternalInput")
    wl0 = nc.dram_tensor("wl0", [128, 128], BF16, kind="ExternalInput")
    wr0 = nc.dram_tensor("wr0", [128, 128], BF16, kind="ExternalInput")
    wl1 = nc.dram_tensor("wl1", [128, 128], BF16, kind="ExternalInput")
    wr1 = nc.dram_tensor("wr1", [128, 128], BF16, kind="ExternalInput")
    wlm = nc.dram_tensor("wlm", [256, 256], BF16, kind="ExternalInput")
    wrm = nc.dram_tensor("wrm", [256, 256], BF16, kind="ExternalInput")
    wlo = nc.dram_tensor("wlo", [256, 64], BF16, kind="ExternalInput")
    wro = nc.dram_tensor("wro", [256, 64], BF16, kind="ExternalInput")
    b01d = nc.dram_tensor("b01", [1, 256], BF16, kind="ExternalInput")
    bmd = nc.dram_tensor("bm", [1, 256], BF16, kind="ExternalInput")
    bod = nc.dram_tensor("bo", [1, 64], BF16, kind="ExternalInput")
    idxd = nc.dram_tensor("idx", [128, ncalls_cols], I16, kind="ExternalInput")
    dstld = nc.dram_tensor("dstl", [128, tiles_total], BF16, kind="ExternalInput")
    invrd = nc.dram_tensor("invr", [128, SP], BF16, kind="ExternalInput")
    degrd = nc.dram_tensor("degr", [128, SP], BF16, kind="ExternalInput")
    invcd = nc.dram_tensor("invc", [128, NBLK], FP32, kind="ExternalInput")
    outd = nc.dram_tensor("out", [S, DO], FP32, kind="ExternalOutput")

    with tile.TileContext(nc) as tc:
        with (
            tc.tile_pool(name="const", bufs=1) as cp,
            tc.tile_pool(name="acts", bufs=1) as hp,
            tc.tile_pool(name="g", bufs=28) as gp,
            tc.tile_pool(name="oh", bufs=6) as ohp,
            tc.tile_pool(name="xs", bufs=6) as xsp,
            # PSUM budget (8 banks): ps0/ps1 (3 bufs each = 6 banks) + py (2)
            tc.tile_pool(name="ps", bufs=3, space="PSUM") as psp,
            tc.tile_pool(name="psy", bufs=2, space="PSUM") as psyp,
            tc.tile_pool(name="ev", bufs=6) as evp,
            tc.tile_pool(name="dram", bufs=1, space="DRAM") as dp,
        ):
            def load(name, dt_, shape, src):
                t = cp.tile(shape, dt_, name=name)
                nc.sync.dma_start(out=t[:], in_=src)
                return t

            wl0t = load("wl0t", BF16, [128, 128], wl0[:])
            wr0t = load("wr0t", BF16, [128, 128], wr0[:])
            wl1t = load("wl1t", BF16, [128, 128], wl1[:])
            wr1t = load("wr1t", BF16, [128, 128], wr1[:])
            wlmt = [load(f"wlmt{i}", BF16, [128, 256], wlm[i * 128:(i + 1) * 128, :]) for i in range(2)]
            wrmt = [load(f"wrmt{i}", BF16, [128, 256], wrm[i * 128:(i + 1) * 128, :]) for i in range(2)]
            wlot = [load(f"wlot{i}", BF16, [128, 64], wlo[i * 128:(i + 1) * 128, :]) for i in range(2)]
            wrot = [load(f"wrot{i}", BF16, [128, 64], wro[i * 128:(i + 1) * 128, :]) for i in range(2)]
            b01t = load("b01t", BF16, [1, 256], b01d[:])
            bmt = load("bmt", BF16, [1, 256], bmd[:])
            bot = load("bot", BF16, [1, 64], bod[:])
            idxt = load("idxt", I16, [128, ncalls_cols], idxd[:])
            invr = load("invrt", BF16, [128, SP], invrd[:])
            degr = load("degrt", BF16, [128, SP], degrd[:])
            invc = load("invct", FP32, [128, NBLK], invcd[:])
            dstl = load("dstlt", BF16, [128, tiles_total], dstld[:])

            iota_i = cp.tile([128, MT, 128], mybir.dt.int32, name="iota_i")
            nc.gpsimd.iota(iota_i[:], pattern=[[0, MT], [1, 128]], base=0,
                           channel_multiplier=0)
            iota_bf = cp.tile([128, MT, 128], BF16, name="iota_bf")
            nc.vector.tensor_copy(out=iota_bf[:], in_=iota_i[:])

            # memset gather pool once: padded idx slots gather row 0 (finite),
            # keeping every slot's stale data finite for zero one-hot columns.
            for i in range(28):
                gz = gp.tile([128, TPC, D1], F8, name="gz", tag="g")
                nc.vector.memset(gz[:], 0.0)

            warm_own = dp.tile([8, 256], F8, name="warm_own")
            warm_tab = dp.tile([64, 256], F8, name="warm_tab",
                               addr_space="Shared" if NCORES > 4 else "Local")
            wz = evp.tile([8, 256], F8, name="wz", tag="wz")
            nc.vector.memset(wz[:], 0.0)
            nc.sync.dma_start(out=warm_own[:], in_=wz[:])
            nc.gpsimd.collective_compute(
                "AllGather", AL.bypass, replica_groups=[list(range(NCORES))],
                ins=[warm_own[:]], outs=[warm_tab[:]])

            hT = [hp.tile([128, SP], BF16, name=f"hT{i}") for i in range(2)]
            h2T = [hp.tile([128, SP], BF16, name=f"h2T{i}") for i in range(2)]

            shared = "Shared" if NCORES > 4 else "Local"

            def mk_tables(name, width):
                own = [dp.tile([CSZ[h], width], F8, name=f"{name}_own{h}")
                       for h in range(NCH)]
                tab = [dp.tile([TBL[h], width], F8, name=f"{name}{h}",
                               addr_space=shared) for h in range(NCH)]
                return own, tab

            y01_own, Y01 = mk_tables("y01", D1)
            ym_own, Ym = mk_tables("ym", DM)
            yo_own, Yo = mk_tables("yo", 256)

            def chunk_of_block(b):
                return 0 if b < CBLK[1] else (1 if b < CBLK[2] else 2)

            def write_y(dsts, b, src_tile, dcols):
                h = chunk_of_block(b)
                r0 = b * 128 - CST[h]
                nc.sync.dma_start(out=dsts[h][r0:r0 + 128, 0:dcols],
                                  in_=src_tile[:, 0:dcols])

            RG = [list(range(NCORES))]

            def blk_sl(b):
                return slice(b * 128, (b + 1) * 128)

            def make_ags(own, tab):
                def mk(h):
                    def f():
                        nc.gpsimd.collective_compute(
                            "AllGather", AL.bypass, replica_groups=RG,
                            ins=[own[h][:]], outs=[tab[h][:]])
                    return f
                return [mk(h) for h in range(NCH)]

            def load_oh(b):
                """Build this block's one-hot tiles with a single DVE IS_EQ."""
                tiles = {}
                for h in range(NCH):
                    start, nt = blk_oh[b][h]
                    t = ohp.tile([128, MT, 128], F8, name=f"ohb{h}", tag=f"oh{h}")
                    nc.vector.tensor_tensor(
                        out=t[:, 0:nt, :], in0=iota_bf[:, 0:nt, :],
                        in1=dstl[:, start:start + nt].to_broadcast([128, nt, 128]),
                        op=AL.is_equal)
                    tiles[h] = (t, start)
                return tiles

            # AG kick: chunk i kicks a few blocks after its rows are written
            # so the kick's input-wait is already satisfied; tail at loop end.
            ag_at = {CBLK[1] + 3: 0, CBLK[2] + 3: 1, CBLK[3] - 1: 2}

            # ================= L1 pre: y01_own = [x0@Wl0 | x1@Wl1] =========
            ags01 = make_ags(y01_own, Y01)
            ag_at_pre = {CBLK[1] - 1: 0, CBLK[2] - 1: 1, CBLK[3] - 1: 2}
            for b in range(NBLK):
                x0b = xsp.tile([128, 128], BF16, name="x0b", tag="x0b")
                nc.sync.dma_start(out=x0b[:], in_=x0T[:, blk_sl(b)])
                x1b = xsp.tile([128, 128], BF16, name="x1b", tag="x1b")
                nc.sync.dma_start(out=x1b[:], in_=x1T[:, blk_sl(b)])
                py0 = psp.tile([128, 128], FP32, name="py0", tag="ps0")
                py1 = psp.tile([128, 128], FP32, name="py1", tag="ps1")
                nc.tensor.matmul(py0[:], lhsT=x0b[:], rhs=wl0t[:], start=True, stop=True)
                nc.tensor.matmul(py1[:], lhsT=x1b[:], rhs=wl1t[:], start=True, stop=True)
                evy = evp.tile([128, 256], F8, name="evy", tag="evy", padded_shape=[128, 512])
                nc.vector.tensor_copy(out=evy[:, 0:128], in_=py0[:])
                nc.vector.tensor_copy(out=evy[:, 128:256], in_=py1[:])
                write_y(y01_own, b, evy, D1)
                if b in ag_at_pre:
                    ags01[ag_at_pre[b]]()

            # ================= aggregation layer (L1/L2) =====================
            def agg_layer(Ytab, wr_tiles, bias_t, h_src, h_dst, wl_next, y_next,
                          d_next, ags_next):
                gtiles = {}
                qn = [0]

                def emit_gathers(cis):
                    for ci in cis:
                        cl = calls[ci]
                        k = cl["k"]
                        g = gp.tile([128, TPC, D1], F8, name="g", tag="g")
                        nc.gpsimd.dma_gather(
                            out_ap=g[:, 0:k, :],
                            in_ap=Ytab[cl["h"]][:],
                            idxs_ap=idxt[:, ci * CPC: ci * CPC + (k * 128) // 16],
                            num_idxs=k * 128, num_idxs_reg=k * 128,
                            elem_size=D1, queue_num=cl["q"])
                        gtiles[ci] = g

                W = 7
                for b in range(W):
                    emit_gathers(block_calls[b][0])
                    emit_gathers(block_calls[b][1])
                for b in range(NBLK):
                    emit_gathers(block_calls[b][2])
                    if b + W < NBLK:
                        emit_gathers(block_calls[b + W][0])
                        emit_gathers(block_calls[b + W][1])
                    ohb = load_oh(b)
                    ps0 = psp.tile([128, 128], FP32, name="ps0", tag="ps0")
                    ps1 = psp.tile([128, 128], FP32, name="ps1", tag="ps1")
                    if h_src is None:
                        x0b = xsp.tile([128, 128], BF16, name="x0b2", tag="xd0")
                        nc.sync.dma_start(out=x0b[:], in_=x0dT[:, blk_sl(b)])
                        x1b = xsp.tile([128, 128], BF16, name="x1b2", tag="xd1")
                        nc.sync.dma_start(out=x1b[:], in_=x1dT[:, blk_sl(b)])
                        nc.tensor.matmul(ps0[:], lhsT=wr0t[:], rhs=x0b[:], start=True, stop=False)
                        nc.tensor.matmul(ps1[:], lhsT=wr1t[:], rhs=x1b[:], start=True, stop=False)
                    else:
                        hd0 = evp.tile([128, 128], BF16, name="hd0", tag="hd0")
                        nc.vector.tensor_tensor(out=hd0[:], in0=h_src[0][:, blk_sl(b)],
                                                in1=degr[:, blk_sl(b)], op=AL.mult)
                        hd1 = evp.tile([128, 128], BF16, name="hd1", tag="hd1")
                        nc.vector.tensor_tensor(out=hd1[:], in0=h_src[1][:, blk_sl(b)],
                                                in1=degr[:, blk_sl(b)], op=AL.mult)
                        nc.tensor.matmul(ps0[:], lhsT=wr_tiles[0][:, 0:128], rhs=hd0[:], start=True, stop=False)
                        nc.tensor.matmul(ps0[:], lhsT=wr_tiles[1][:, 0:128], rhs=hd1[:], start=False, stop=False)
                        nc.tensor.matmul(ps1[:], lhsT=wr_tiles[0][:, 128:256], rhs=hd0[:], start=True, stop=False)
                        nc.tensor.matmul(ps1[:], lhsT=wr_tiles[1][:, 128:256], rhs=hd1[:], start=False, stop=False)
                    nc.tensor.matmul(ps0[:], lhsT=bias_t[0:1, 0:128], rhs=degr[0:1, blk_sl(b)],
                                     start=False, stop=False)
                    nc.tensor.matmul(ps1[:], lhsT=bias_t[0:1, 128:256], rhs=degr[0:1, blk_sl(b)],
                                     start=False, stop=False)
                    tl = block_tiles[b]
                    for n, (ci, slot, dcol) in enumerate(tl):
                        g = gtiles[ci]
                        oht, start = ohb[calls[ci]["h"]]
                        j = dcol - start
                        last = (n == len(tl) - 1)
                        nc.tensor.matmul(ps0[:], lhsT=g[:, slot, 0:128], rhs=oht[:, j, :],
                                         start=False, stop=last)
                        nc.tensor.matmul(ps1[:], lhsT=g[:, slot, 128:256], rhs=oht[:, j, :],
                                         start=False, stop=last)
                    # epilogue: h = relu(ps) * invd  (relu commutes with the
                    # positive per-column scale)
                    rt0 = evp.tile([128, 128], BF16, name="rt0", tag="rt0")
                    nc.scalar.activation(rt0[:], ps0[:], AF.Relu)
                    nc.vector.tensor_tensor(out=h_dst[0][:, blk_sl(b)], in0=rt0[:],
                                            in1=invr[:, blk_sl(b)], op=AL.mult)
                    rt1 = evp.tile([128, 128], BF16, name="rt1", tag="rt1")
                    nc.scalar.activation(rt1[:], ps1[:], AF.Relu)
                    nc.vector.tensor_tensor(out=h_dst[1][:, blk_sl(b)], in0=rt1[:],
                                            in1=invr[:, blk_sl(b)], op=AL.mult)
                    pyn = psyp.tile([128, d_next], FP32, name="pyn", tag="py",
                                    padded_shape=[128, 256])
                    nc.tensor.matmul(pyn[:], lhsT=h_dst[0][:, blk_sl(b)], rhs=wl_next[0][:],
                                     start=True, stop=False)
                    nc.tensor.matmul(pyn[:], lhsT=h_dst[1][:, blk_sl(b)], rhs=wl_next[1][:],
                                     start=False, stop=True)
                    evn = evp.tile([128, d_next], F8, name="evn", tag="evy",
                                   padded_shape=[128, 512])
                    nc.vector.tensor_copy(out=evn[:], in_=pyn[:])
                    write_y(y_next, b, evn, d_next)
                    if b in ag_at:
                        ags_next[ag_at[b]]()

            agg_layer(Y01, None, b01t, None, hT, wlmt, ym_own, DM,
                      make_ags(ym_own, Ym))
            agg_layer(Ym, wrmt, bmt, hT, h2T, wlot, yo_own, DO,
                      make_ags(yo_own, Yo))

            # ================= L3: out[node, 64] ============================
            qn3 = [0]
            gtiles3 = {}

            def emit_gathers3(cis):
                for ci in cis:
                    cl = calls[ci]
                    k = cl["k"]
                    g3 = gp.tile([128, TPC, 256], F8, name="g3", tag="g")
                    nc.gpsimd.dma_gather(
                        out_ap=g3[:, 0:k, :], in_ap=Yo[cl["h"]][:],
                        idxs_ap=idxt[:, ci * CPC: ci * CPC + (k * 128) // 16],
                        num_idxs=k * 128, num_idxs_reg=k * 128,
                        elem_size=256, queue_num=cl["q"])
                    gtiles3[ci] = g3

            W3 = 7
            for b in range(W3):
                emit_gathers3(block_calls[b][0])
                emit_gathers3(block_calls[b][1])
            for b in range(NBLK):
                emit_gathers3(block_calls[b][2])
                if b + W3 < NBLK:
                    emit_gathers3(block_calls[b + W3][0])
                    emit_gathers3(block_calls[b + W3][1])
                ohb = load_oh(b)
                ps3 = psp.tile([128, DO], FP32, name="ps3", tag="ps0",
                               padded_shape=[128, 128])
                hd0 = evp.tile([128, 128], BF16, name="hd20", tag="hd0")
                nc.vector.tensor_tensor(out=hd0[:], in0=h2T[0][:, blk_sl(b)],
                                        in1=degr[:, blk_sl(b)], op=AL.mult)
                hd1 = evp.tile([128, 128], BF16, name="hd21", tag="hd1")
                nc.vector.tensor_tensor(out=hd1[:], in0=h2T[1][:, blk_sl(b)],
                                        in1=degr[:, blk_sl(b)], op=AL.mult)
                nc.tensor.matmul(ps3[:], lhsT=hd0[:], rhs=wrot[0][:],
                                 start=True, stop=False)
                nc.tensor.matmul(ps3[:], lhsT=hd1[:], rhs=wrot[1][:],
                                 start=False, stop=False)
                nc.tensor.matmul(ps3[:], lhsT=degr[0:1, blk_sl(b)], rhs=bot[0:1, :],
                                 start=False, stop=False)
                tl = block_tiles[b]
                for n, (ci, slot, dcol) in enumerate(tl):
                    g3 = gtiles3[ci]
                    oht, start = ohb[calls[ci]["h"]]
                    j = dcol - start
                    nc.tensor.matmul(ps3[:], lhsT=oht[:, j, :], rhs=g3[:, slot, 0:64],
                                     start=False, stop=(n == len(tl) - 1))
                osb = evp.tile([128, DO], FP32, name="osb", tag="osb")
                nc.scalar.activation(osb[:], ps3[:], AF.Copy,
                                     scale=invc[:, b:b + 1])
                rows = min(128, S - b * 128)
                nc.sync.dma_start(out=outd[b * 128: b * 128 + rows, :],
                                  in_=osb[0:rows, :])

    nc.finalize()
    return nc


_CACHE = {}


def _make_inmaps(inputs, pre, calls):
    import ml_dtypes as _ml
    BF = _ml.bfloat16
    x0 = np.asarray(inputs["x0"], np.float32)
    x1 = np.asarray(inputs["x1"], np.float32)
    deg = pre["deg"]
    bf16 = lambda a: np.ascontiguousarray(a).astype(BF)
    in_maps = []
    for c in range(NCORES):
        degc = np.maximum(deg[c], 1.0).astype(np.float32)
        invd = (1.0 / degc).astype(np.float32)
        degc_p = np.ones(SP, np.float32)
        degc_p[:S] = degc
        invd_p = np.ones(SP, np.float32)
        invd_p[:S] = invd
        idx_img, dstloc = _idx_arrays(pre, calls, c)
        x0c = np.zeros((128, SP), np.float32)
        x0c[:, :S] = x0[c * S:(c + 1) * S, :].T
        x1c = np.zeros((128, SP), np.float32)
        x1c[:, :S] = x1[c * S:(c + 1) * S, :].T
        x0dc = x0c * degc_p[None, :]
        x1dc = x1c * degc_p[None, :]
        in_maps.append({
            "x0T": bf16(x0c), "x1T": bf16(x1c),
            "x0dT": bf16(x0dc), "x1dT": bf16(x1dc),
            "wl0": bf16(inputs["Wl0"]), "wr0": bf16(inputs["Wr0"]),
            "wl1": bf16(inputs["Wl1"]), "wr1": bf16(inputs["Wr1"]),
            "wlm": bf16(inputs["Wlm"]), "wrm": bf16(inputs["Wrm"]),
            "wlo": bf16(inputs["Wlo"]), "wro": bf16(inputs["Wro"]),
            "b01": bf16(np.concatenate([np.asarray(inputs["b0"], np.float32),
                                        np.asarray(inputs["b1"], np.float32)])[None, :]),
            "bm": bf16(np.asarray(inputs["bm"], np.float32)[None, :]),
            "bo": bf16(np.asarray(inputs["bo"], np.float32)[None, :]),
            "idx": idx_img, "dstl": dstloc,
            "invr": bf16(np.broadcast_to(invd_p[None, :], (128, SP))),
            "degr": bf16(np.broadcast_to(degc_p[None, :], (128, SP))),
            "invc": np.ascontiguousarray(invd_p[:NBLK * 128].reshape(NBLK, 128).T,
                                         np.float32),
        })
    return in_maps


def _get_program(edge_index):
    if "prog" in _CACHE:
        return _CACHE["prog"]
    pre = _preprocess(edge_index)
    calls, block_calls, block_tiles = _build_callplan(pre["tiles_hb"])
    tiles_total = int(pre["tiles_hb"].sum())
    nc = _build_bass(pre, calls, block_calls, block_tiles, len(calls) * CPC, tiles_total)
    _CACHE["prog"] = (nc, pre, calls)
    return _CACHE["prog"]


LAST_EXEC_NS = None


def kernel(**inputs):
    global LAST_EXEC_NS
    _install_hooks()
    from concourse.bass_utils import run_bass_kernel_spmd

    nc, pre, calls = _get_program(inputs["edge_index"])
    in_maps = _make_inmaps(inputs, pre, calls)
    trace = os.environ.get("KERNEL_TRACE", "0") == "1"
    res = run_bass_kernel_spmd(nc, in_maps, list(range(NCORES)), trace=trace)
    LAST_EXEC_NS = res.exec_time_ns
    return np.concatenate([np.asarray(res.results[c]["out"]) for c in range(NCORES)], axis=0)
